# revision 7
# baseline (speedup 1.0000x reference)
"""Trainium2 Bass kernel for a multi-head self-attention block.

Reference computation (B=4, N=2048, D=256, H=8, dh=32, DFF=512):
    x_ln = LN0(x); Q = x_ln@Wq.T+bq; K = y@Wk.T+bk; V = y@Wv.T+bv
    per head: A = softmax(Qh Kh^T / 16); O = concat_h(Qh + A Vh)
    out = O + (gelu(LN1(O)@W1.T+b1) @ W2.T + b2)

Sharding: 8 cores = 4 batches x 2 halves of the query sequence. Each core
gets its x half-shard and the full y for its batch; no collectives.

On-chip layout is feature-on-partition ("transposed"): X^T/Q^T/K^T/O^T are
[D, tokens] so every matmul contraction lands on partitions with zero
transposes. Head h lives at partition strip 32*(h%4), ktile o=h//4.
Scores are computed transposed (S^T[k,q] tiles), exp runs on ScalarE with
the 1/16 scale folded in (no max-subtraction: |scores/16| < ~1.5), row
sums come from ones-vector matmuls accumulated alongside AV, and the
softmax division is applied to the small AV output, not the NxN matrix.
Biases are added via K=1 matmuls against an all-ones tile.
"""

import contextlib

import numpy as np

B, N, D = 4, 2048, 256
H, DH, DFF = 8, 32, 512
P = 128
NTOK = N // 2            # query tokens per core
NQT = NTOK // 512        # q tiles of 512
NKT = N // P             # key tiles of 128
SCALE = 1.0 / 16.0
EPS = 1e-5

_NC_CACHE = {}


def _build_nc():
    import concourse.mybir as mybir
    import concourse.tile as tile
    from concourse import bacc

    f32 = mybir.dt.float32
    AF = mybir.ActivationFunctionType
    ALU = mybir.AluOpType

    nc = bacc.Bacc("TRN2", target_bir_lowering=False, debug=False)

    xt_d = nc.dram_tensor("xt", [D, NTOK], f32, kind="ExternalInput")
    yt_d = nc.dram_tensor("yt", [D, N], f32, kind="ExternalInput")
    wq_d = nc.dram_tensor("wq", [D + 1, D], f32, kind="ExternalInput")
    wk_d = nc.dram_tensor("wk", [D + 1, D], f32, kind="ExternalInput")
    wv_d = nc.dram_tensor("wv", [D + 1, D], f32, kind="ExternalInput")
    w1_d = nc.dram_tensor("w1", [D + 1, DFF], f32, kind="ExternalInput")
    w2_d = nc.dram_tensor("w2", [DFF + 1, D], f32, kind="ExternalInput")
    out_d = nc.dram_tensor("out_t", [D, NTOK], f32, kind="ExternalOutput")

    with tile.TileContext(nc) as tc, contextlib.ExitStack() as ctx:
        const = ctx.enter_context(tc.tile_pool(name="const", bufs=1))
        big = ctx.enter_context(tc.tile_pool(name="big", bufs=1))
        scratch = ctx.enter_context(tc.tile_pool(name="scratch", bufs=1))
        apool = ctx.enter_context(tc.tile_pool(name="apool", bufs=3))
        # PSUM: scores 2x[128,1024]=4 banks, av/sm/bc 1 bank each, proj 1.
        scores_pool = ctx.enter_context(
            tc.tile_pool(name="scoresp", bufs=2, space="PSUM"))
        av_pool = ctx.enter_context(tc.tile_pool(name="avp", bufs=1, space="PSUM"))
        sm_pool = ctx.enter_context(tc.tile_pool(name="smp", bufs=1, space="PSUM"))
        bc_pool = ctx.enter_context(tc.tile_pool(name="bcp", bufs=1, space="PSUM"))
        proj_pool = ctx.enter_context(tc.tile_pool(name="projp", bufs=1, space="PSUM"))

        def sc_tile():
            return scores_pool.tile([P, 1024], f32, tag="scores", name="sc")

        # ---- constants / weights -------------------------------------------
        ones_s = const.tile([P, 512], f32)
        nc.vector.memset(ones_s[:], 1.0)
        eps_s = const.tile([1, 1], f32)
        nc.vector.memset(eps_s[:], EPS)

        xt_s = big.tile([P, 2, NTOK], f32)
        nc.sync.dma_start(xt_s[:], xt_d.rearrange("(o p) t -> p o t", p=P))
        yt_s = big.tile([P, 2, N], f32)
        nc.sync.dma_start(yt_s[:], yt_d.rearrange("(o p) t -> p o t", p=P))

        wq_s = const.tile([P, 3, D], f32)
        wk_s = const.tile([P, 3, D], f32)
        wv_s = const.tile([P, 3, D], f32)
        w1_s = const.tile([P, 3, DFF], f32)
        w2_s = const.tile([P, 5, D], f32)
        for wsb, wdr, kt_full in ((wq_s, wq_d, 2), (wk_s, wk_d, 2),
                                  (wv_s, wv_d, 2), (w1_s, w1_d, 2),
                                  (w2_s, w2_d, 4)):
            nc.sync.dma_start(
                wsb[:, 0:kt_full, :],
                wdr[0:kt_full * P, :].rearrange("(o p) m -> p o m", p=P))
            nc.sync.dma_start(wsb[0:1, kt_full, :], wdr[kt_full * P:, :])

        # ---- helper: layernorm over the partition-tiled feature dim --------
        def layernorm(src, dst, ntok):
            """src/dst: [128, 2, ntok] sbuf tiles; normalizes over the 256
            feature rows per token column (no affine)."""
            sq = scratch.tile([P, 2, ntok], f32, tag="sq")
            nc.scalar.activation(out=sq[:], in_=src[:], func=AF.Square)
            mean = scratch.tile([1, ntok], f32, tag="mean")
            rstd = scratch.tile([1, ntok], f32, tag="rstd")
            tmp = scratch.tile([1, ntok], f32, tag="lntmp")
            for hf in range(ntok // 512):
                cs = slice(hf * 512, hf * 512 + 512)
                sx_ps = av_pool.tile([1, 512], f32, tag="av")
                sq_ps = sm_pool.tile([1, 512], f32, tag="sm")
                for o in range(2):
                    nc.tensor.matmul(sx_ps[:], lhsT=ones_s[:, 0:1],
                                     rhs=src[:, o, cs],
                                     start=(o == 0), stop=(o == 1))
                    nc.tensor.matmul(sq_ps[:], lhsT=ones_s[:, 0:1],
                                     rhs=sq[:, o, cs],
                                     start=(o == 0), stop=(o == 1))
                nc.vector.tensor_scalar_mul(mean[0:1, cs], sx_ps[:], 1.0 / D)
                nc.vector.tensor_scalar_mul(tmp[0:1, cs], sq_ps[:], 1.0 / D)
            # var = E[x^2] - mean^2 ; rstd = 1/sqrt(var + eps)
            m2 = scratch.tile([1, ntok], f32, tag="m2")
            nc.vector.tensor_tensor(out=m2[:], in0=mean[:], in1=mean[:],
                                    op=ALU.mult)
            nc.vector.tensor_tensor(out=tmp[:], in0=tmp[:], in1=m2[:],
                                    op=ALU.subtract)
            nc.scalar.activation(out=tmp[:], in_=tmp[:], func=AF.Sqrt,
                                 bias=eps_s[:])
            nc.vector.reciprocal(out=rstd[:], in_=tmp[:])
            # broadcast mean/rstd to all partitions via PE, then normalize
            meanb = sc_tile()
            rstdb = sc_tile()
            for hf in range(ntok // 512):
                cs = slice(hf * 512, hf * 512 + 512)
                nc.tensor.matmul(meanb[:, cs], lhsT=ones_s[0:1, 0:P],
                                 rhs=mean[0:1, cs], start=True, stop=True)
                nc.tensor.matmul(rstdb[:, cs], lhsT=ones_s[0:1, 0:P],
                                 rhs=rstd[0:1, cs], start=True, stop=True)
            ntmp = scratch.tile([P, 2, ntok], f32, tag="sq")
            for o in range(2):
                nc.vector.tensor_tensor(out=ntmp[:, o, :], in0=src[:, o, :],
                                        in1=meanb[:], op=ALU.subtract)
                nc.vector.tensor_tensor(out=dst[:, o, :], in0=ntmp[:, o, :],
                                        in1=rstdb[:], op=ALU.mult)

        # ---- helper: projection out^T[dout, tok] = W^T.T @ in^T + bias -----
        def project(w_s, nkt_w, rhs_fn, ntiles_n, nsz, out_fn):
            """w_s: [128, nkt_w+1, dout] weights (+bias row at [0:1, nkt_w, :]).
            rhs_fn(o, nt) -> rhs AP [128, nsz] (ktile o, col tile nt).
            out_fn(mt, nt) -> sbuf dest AP [128, nsz]."""
            dout = w_s.shape[2]
            for mt in range(dout // P):
                ms = slice(mt * P, mt * P + P)
                for nt in range(ntiles_n):
                    ps = proj_pool.tile([P, 512], f32, tag="proj", name="ps")[:, 0:nsz]
                    for o in range(nkt_w):
                        nc.tensor.matmul(ps[:], lhsT=w_s[:, o, ms],
                                         rhs=rhs_fn(o, nt),
                                         start=(o == 0), stop=False)
                    nc.tensor.matmul(ps[:], lhsT=w_s[0:1, nkt_w, ms],
                                     rhs=ones_s[0:1, 0:nsz],
                                     start=False, stop=True)
                    nc.vector.tensor_copy(out=out_fn(mt, nt), in_=ps[:])

        # ---- phase A: LN0, Q/K/V projections -------------------------------
        xln_s = big.tile([P, 2, NTOK], f32)
        layernorm(xt_s, xln_s, NTOK)

        qt_s = big.tile([P, 2, NTOK], f32)
        project(wq_s, 2, lambda o, nt: xln_s[:, o, nt * 512:nt * 512 + 512],
                NTOK // 512, 512,
                lambda mt, nt: qt_s[:, mt, nt * 512:nt * 512 + 512])
        kt_s = big.tile([P, 2, N], f32)
        project(wk_s, 2, lambda o, nt: yt_s[:, o, nt * 512:nt * 512 + 512],
                N // 512, 512,
                lambda mt, nt: kt_s[:, mt, nt * 512:nt * 512 + 512])
        # V in natural [token, dout] layout: lhsT = y^T tok-tile, rhs = W_v^T
        v_s = big.tile([P, NKT, D], f32)
        for tt in range(NKT):
            ts_ = slice(tt * P, tt * P + P)
            ps = proj_pool.tile([P, 512], f32, tag="proj", name="ps")[:, 0:D]
            for o in range(2):
                nc.tensor.matmul(ps[:], lhsT=yt_s[:, o, ts_],
                                 rhs=wv_s[:, o, :], start=(o == 0), stop=False)
            nc.tensor.matmul(ps[:], lhsT=ones_s[0:1, 0:P],
                             rhs=wv_s[0:1, 2, :], start=False, stop=True)
            nc.vector.tensor_copy(out=v_s[:, tt, :], in_=ps[:])

        # ---- phase B: attention -------------------------------------------
        ot_s = big.tile([P, 2, NTOK], f32)
        rc_s = scratch.tile([P, 512], f32, tag="rc")
        for hg in range(2):
            for qt in range(NQT):
                qs_ = slice(qt * 512, qt * 512 + 512)
                av = av_pool.tile([P, 512], f32, tag="av")
                sm = sm_pool.tile([P, 512], f32, tag="sm")
                for kt in range(NKT):
                    ks_ = slice(kt * P, kt * P + P)
                    for pair in range(2):
                        sp = sc_tile()
                        for jj in range(2):
                            j = 2 * pair + jj
                            js = slice(32 * j, 32 * j + 32)
                            nc.tensor.matmul(
                                sp[:, jj * 512:jj * 512 + 512],
                                lhsT=kt_s[js, hg, ks_],
                                rhs=qt_s[js, hg, qs_],
                                start=True, stop=True,
                                tile_position=(32 * j, 0))
                        a = apool.tile([P, 1024], f32, tag="a")
                        nc.scalar.activation(out=a[:], in_=sp[:], func=AF.Exp,
                                             scale=SCALE)
                        for jj in range(2):
                            j = 2 * pair + jj
                            h = 4 * hg + j
                            nc.tensor.matmul(
                                av[32 * j:32 * j + 32, :],
                                lhsT=v_s[:, kt, 32 * h:32 * h + 32],
                                rhs=a[:, jj * 512:jj * 512 + 512],
                                start=(kt == 0), stop=(kt == NKT - 1),
                                tile_position=(0, 32 * j),
                                skip_group_check=True)
                            nc.tensor.matmul(
                                sm[32 * j:32 * j + 1, :],
                                lhsT=ones_s[:, 0:1],
                                rhs=a[:, jj * 512:jj * 512 + 512],
                                start=(kt == 0), stop=(kt == NKT - 1),
                                tile_position=(0, 32 * j),
                                skip_group_check=True)
                # normalize by row sums + per-head residual with Q
                bc = bc_pool.tile([P, 512], f32, tag="bc")
                for j in range(4):
                    nc.vector.reciprocal(out=rc_s[32 * j:32 * j + 1, :],
                                         in_=sm[32 * j:32 * j + 1, :])
                    nc.tensor.matmul(bc[32 * j:32 * j + 32, :],
                                     lhsT=ones_s[32 * j:32 * j + 1, 0:32],
                                     rhs=rc_s[32 * j:32 * j + 1, :],
                                     start=True, stop=True,
                                     tile_position=(32 * j, 32 * j))
                av_sb = scratch.tile([P, 512], f32, tag="avsb")
                nc.vector.tensor_copy(out=av_sb[:], in_=av[:])
                nrm = scratch.tile([P, 512], f32, tag="nrm")
                nc.vector.tensor_tensor(out=nrm[:], in0=av_sb[:], in1=bc[:],
                                        op=ALU.mult)
                nc.vector.tensor_tensor(out=ot_s[:, hg, qs_], in0=nrm[:],
                                        in1=qt_s[:, hg, qs_], op=ALU.add)

        # ---- phase C: LN1 + FFN + final residual ---------------------------
        oln_s = big.tile([P, 2, NTOK], f32)
        layernorm(ot_s, oln_s, NTOK)

        h_s = big.tile([P, 4, NTOK], f32)
        for mt in range(DFF // P):
            ms = slice(mt * P, mt * P + P)
            for nt in range(NQT):
                ns_ = slice(nt * 512, nt * 512 + 512)
                ps = proj_pool.tile([P, 512], f32, tag="proj", name="ps")
                for o in range(2):
                    nc.tensor.matmul(ps[:], lhsT=w1_s[:, o, ms],
                                     rhs=oln_s[:, o, ns_],
                                     start=(o == 0), stop=False)
                nc.tensor.matmul(ps[:], lhsT=w1_s[0:1, 2, ms],
                                 rhs=ones_s[0:1, 0:512], start=False, stop=True)
                nc.scalar.activation(out=h_s[:, mt, ns_], in_=ps[:],
                                     func=AF.Gelu)

        outt_s = big.tile([P, 2, NTOK], f32)
        out_r = out_d.rearrange("(o p) t -> p o t", p=P)
        for mt in range(D // P):
            ms = slice(mt * P, mt * P + P)
            for nt in range(NQT):
                ns_ = slice(nt * 512, nt * 512 + 512)
                ps = proj_pool.tile([P, 512], f32, tag="proj", name="ps")
                for o in range(4):
                    nc.tensor.matmul(ps[:], lhsT=w2_s[:, o, ms],
                                     rhs=h_s[:, o, ns_],
                                     start=(o == 0), stop=False)
                nc.tensor.matmul(ps[:], lhsT=w2_s[0:1, 4, ms],
                                 rhs=ones_s[0:1, 0:512], start=False, stop=True)
                nc.vector.tensor_tensor(out=outt_s[:, mt, ns_], in0=ps[:],
                                        in1=ot_s[:, mt, ns_], op=ALU.add)
                nc.sync.dma_start(out_r[:, mt, ns_], outt_s[:, mt, ns_])

    nc.compile()
    return nc


def get_nc():
    if "nc" not in _NC_CACHE:
        _NC_CACHE["nc"] = _build_nc()
    return _NC_CACHE["nc"]


def _host_prep(inputs):
    f = lambda k: np.asarray(inputs[k], np.float32)
    x, y = f("x"), f("y")
    Wq, bq, Wk, bk, Wv, bv = f("Wq"), f("bq"), f("Wk"), f("bk"), f("Wv"), f("bv")
    W1, b1, W2, b2 = f("W1"), f("b1"), f("W2"), f("b2")
    ln0_g, ln0_b, ln1_g, ln1_b = f("ln0_g"), f("ln0_b"), f("ln1_g"), f("ln1_b")
    # fold LN affine params into the following linear layers
    Wq_eff = Wq * ln0_g[None, :]
    bq_eff = bq + Wq @ ln0_b
    W1_eff = W1 * ln1_g[None, :]
    b1_eff = b1 + W1 @ ln1_b
    wq_h = np.ascontiguousarray(np.vstack([Wq_eff.T, bq_eff[None, :]]))
    wk_h = np.ascontiguousarray(np.vstack([Wk.T, bk[None, :]]))
    wv_h = np.ascontiguousarray(np.vstack([Wv.T, bv[None, :]]))
    w1_h = np.ascontiguousarray(np.vstack([W1_eff.T, b1_eff[None, :]]))
    w2_h = np.ascontiguousarray(np.vstack([W2.T, b2[None, :]]))
    in_maps = []
    for core in range(8):
        b, half = core // 2, core % 2
        in_maps.append({
            "xt": np.ascontiguousarray(x[b, half * NTOK:(half + 1) * NTOK, :].T),
            "yt": np.ascontiguousarray(y[b].T),
            "wq": wq_h, "wk": wk_h, "wv": wv_h, "w1": w1_h, "w2": w2_h,
        })
    return in_maps


def kernel_with_results(inputs, **run_kwargs):
    from concourse.bass_utils import run_bass_kernel_spmd
    nc = get_nc()
    in_maps = _host_prep(inputs)
    res = run_bass_kernel_spmd(nc, in_maps, core_ids=list(range(8)), **run_kwargs)
    out = np.empty((B, N, D), np.float32)
    for core in range(8):
        b, half = core // 2, core % 2
        out[b, half * NTOK:(half + 1) * NTOK, :] = res.results[core]["out_t"].T
    return out, res


def kernel(**inputs):
    out, _ = kernel_with_results(inputs)
    return out
